# revision 39
# baseline (speedup 1.0000x reference)
"""Multi-head causal attention with RoPE on 8 Trainium2 NeuronCores.

Sharding: 2 (batch) x 4 (head-groups of 4 heads). Each core computes
QKV projections, RoPE, flash-style causal attention and its slice of the
output projection for one batch and 4 heads; partial outputs are summed
on the host (row-sharded out_proj => partial-sum reduction).

Device layout choices (everything host-prepped to avoid on-device
transposes, fp32 has no DMA-transpose path):
  - x is passed pre-transposed per batch: xT [D, S] bf16
  - Q^T, K^T computed as [head_dim, S] (lhsT = W tile, rhs = xT)
  - V computed natural [S, head_dim] (lhsT = xT tile, rhs = Wv);
    emitted o-major across 4 parallel si-chains so the PE tracks the
    x DMA stream during startup (no long cold-start stall)
  - scores computed transposed [k, q]; softmax sum over k (partitions)
    via gpsimd partition_all_reduce (lands the sum on every partition,
    so normalization needs no broadcast and no PE ones-matmul)
  - RoPE rotate-half via two SBUF->SBUF partition-swap DMAs plus a
    sign-folded sin constant (no PE permutation matmul)
  - causal diagonal k-tiles use width-trimmed score/PV matmuls
    (only the q >= k columns), one 128x128 tril mask for the true
    diagonal block
  - attention g-outer with per-q-group out-proj interleaved
"""

import math
import sys

import numpy as np

try:
    import concourse.bass as bass  # noqa: F401
except Exception:
    sys.path.insert(0, "/opt/trn_rl_repo")

import ml_dtypes

P = 128
B = 2
S = 2048
D = 2048
H = 16
HEAD = 128
N_CORES = 8
HG = 4            # head groups (tensor-parallel dimension)
HPG = H // HG     # heads per group = 4
DG = HPG * HEAD   # group width = 512
SG = 512          # q-group (free dim) size
DOUT = 2048

BF16 = ml_dtypes.bfloat16


def _emit(tc, io, cfg, sfx=""):
    """Emit the per-core program. io: dict of dram APs. cfg: sizes."""
    import concourse.mybir as mybir
    nc = tc.nc
    bf = mybir.dt.bfloat16
    f32 = mybir.dt.float32
    Exp = mybir.ActivationFunctionType.Exp

    s = cfg["S"]
    d = cfg["D"]
    dout = cfg["DOUT"]
    di_t = d // P          # d_in k-tiles
    st = s // P            # seq 128-tiles
    nsg = s // SG          # seq 512-groups
    nos = dout // SG       # out column slices
    inv_sqrt_hd = 1.0 / math.sqrt(HEAD)

    xT = io["xT"].rearrange("(o p) s -> p o s", p=P)
    wq = io["wq"].rearrange("(o p) n -> p o n", p=P)
    wk = io["wk"].rearrange("(o p) n -> p o n", p=P)
    wv = io["wv"].rearrange("(o p) n -> p o n", p=P)
    wo = io["wo"].rearrange("(o p) n -> p o n", p=P)

    const = tc.alloc_tile_pool(name="const" + sfx, bufs=1)
    stores = tc.alloc_tile_pool(name="stores" + sfx, bufs=1)

    # ---- constants (tiles only; DMAs emitted inside the phase-1 stream) ----
    cos_sb = const.tile([P, s], bf, tag="cos")
    sin_sb = const.tile([P, s], bf, tag="sin")     # sign-folded rope sin
    mask_sb = const.tile([P, P], bf, tag="mask")   # tril (k<=q) diagonal mask
    ones_sb = const.tile([P, P], bf, tag="ones")
    wv_sb = const.tile([P, di_t, DG], bf, tag="wv")

    # persistent activation stores
    qt_sb = stores.tile([P, HPG, s], bf, tag="qt")
    kt_sb = stores.tile([P, HPG, s], bf, tag="kt")
    v_sb = stores.tile([P, st, DG], bf, tag="v")
    ctx_sb = stores.tile([P, HPG, s], bf, tag="ctx")

    # ---- phase 1: projections + RoPE ----
    with tc.tile_pool(name="xt" + sfx, bufs=1) as xtp, \
         tc.tile_pool(name="wqk" + sfx, bufs=1) as wqkp, \
         tc.tile_pool(name="p1tmp" + sfx, bufs=5) as p1tmp, \
         tc.tile_pool(name="p1sw" + sfx, bufs=5) as p1sw, \
         tc.tile_pool(name="ps_v" + sfx, bufs=4, space="PSUM") as ps_v, \
         tc.tile_pool(name="ps_qk" + sfx, bufs=4, space="PSUM") as ps_qk:
        xt_sb = xtp.tile([P, di_t, s], bf, tag="xt")
        wq_sb = wqkp.tile([P, di_t, DG], bf, tag="wq")
        wk_sb = wqkp.tile([P, di_t, DG], bf, tag="wk")
        # x streams as full 524KB o-rows (4KB/partition lines -> near-peak
        # DMA bw); weights as a handful of large DMAs interleaved so the
        # issuing-engine cost (~0.6us per dma_start) stays negligible.
        nc.sync.dma_start(wv_sb[:, 0:1, :], wv[:, 0:1, :])
        for o in range(di_t):
            nc.sync.dma_start(xt_sb[:, o, :], xT[:, o, :])
            if o == 0:
                nc.sync.dma_start(wv_sb[:, 1:4, :], wv[:, 1:4, :])
            elif o % 4 == 3 and o + 1 < di_t:
                c = o + 1
                nc.sync.dma_start(wv_sb[:, c:c + 4, :], wv[:, c:c + 4, :])
        nc.sync.dma_start(cos_sb[:], io["cosT"][:])
        nc.sync.dma_start(sin_sb[:], io["sinT"][:])
        nc.sync.dma_start(mask_sb[:], io["mask"][:])
        nc.sync.dma_start(ones_sb[:], io["ones"][:])
        nc.sync.dma_start(wq_sb[:], wq[:])
        nc.sync.dma_start(wk_sb[:], wk[:])

        # V natural layout [s_tile, DG]. First 8 si-chains run o-major in
        # parallel (all 8 PSUM banks) so the PE tracks the x o-row DMA
        # stream during startup; the rest run chain-at-a-time so
        # evacuations overlap.
        pvs = [
            (ps_v if i < 4 else ps_qk).tile(
                [P, SG], f32, tag="psv" if i < 4 else "ps", name=f"pv0_{i}"
            )
            for i in range(8)
        ]
        for o in range(di_t):
            for q8 in range(8):
                nc.tensor.matmul(
                    pvs[q8][:, :DG],
                    lhsT=xt_sb[:, o, q8 * P:(q8 + 1) * P],
                    rhs=wv_sb[:, o, :],
                    start=(o == 0),
                    stop=(o == di_t - 1),
                )
        for q8 in range(8):
            nc.vector.tensor_copy(v_sb[:, q8, :], pvs[q8][:, :DG])
        for si in range(8, st):
            pv = ps_v.tile([P, SG], f32, tag="psv")
            for o in range(di_t):
                nc.tensor.matmul(
                    pv[:, :DG],
                    lhsT=xt_sb[:, o, si * P:(si + 1) * P],
                    rhs=wv_sb[:, o, :],
                    start=(o == 0),
                    stop=(o == di_t - 1),
                )
            nc.vector.tensor_copy(v_sb[:, si, :], pv[:, :DG])

        # Q^T, K^T with RoPE, per head. Rotate-half = partition swap via
        # two SBUF->SBUF DMAs + sign-folded sin; pipelined 2 deep so the
        # PE never waits on the ACT psum->sbuf copy or the swap DMA.
        def emit_rope(qa, qsw, dst, hh, sl):
            t1 = p1tmp.tile([P, SG], bf, tag="t1")
            nc.vector.tensor_mul(t1, qa, cos_sb[:, sl])
            t2 = p1tmp.tile([P, SG], bf, tag="t2")
            nc.vector.tensor_mul(t2, qsw, sin_sb[:, sl])
            nc.vector.tensor_add(dst[:, hh, sl], t1, t2)

        pending = []
        for h in range(HPG):
            hsl = slice(h * P, (h + 1) * P)
            for g in range(nsg):
                sl = slice(g * SG, (g + 1) * SG)
                for w_t, dst in ((wq_sb, qt_sb), (wk_sb, kt_sb)):
                    pq = ps_qk.tile([P, SG], f32, tag="ps")
                    for o in range(di_t):
                        nc.tensor.matmul(
                            pq,
                            lhsT=w_t[:, o, hsl],
                            rhs=xt_sb[:, o, sl],
                            start=(o == 0),
                            stop=(o == di_t - 1),
                        )
                    qa = p1tmp.tile([P, SG], bf, tag="qa")
                    nc.scalar.copy(qa, pq)
                    qsw = p1sw.tile([P, SG], bf, tag="qsw")
                    nc.sync.dma_start(qsw[0:64, :], qa[64:128, :])
                    nc.sync.dma_start(qsw[64:128, :], qa[0:64, :])
                    pending.append((qa, qsw, dst, h, sl))
                    while len(pending) > 2:
                        emit_rope(*pending.pop(0))
        while pending:
            emit_rope(*pending.pop(0))

    # ---- phase 2+3: attention interleaved with output projection ----
    # g outer so each q-group's out-proj tiles become ready early and fill
    # the PE while later q-groups' softmax runs. Diagonal k-tiles use
    # width-trimmed matmuls; softmax sums pair-tree on DVE into one tile,
    # then a single full-width ones-matmul per (h, g) lands the sum on
    # every PSUM partition so normalization needs no broadcast.
    ps_sc = tc.alloc_tile_pool(name="ps_sc" + sfx, bufs=2, space="PSUM")
    ps_main = tc.alloc_tile_pool(name="ps_main" + sfx, bufs=3, space="PSUM")
    ps_l = tc.alloc_tile_pool(name="ps_l" + sfx, bufs=1, space="PSUM")
    with tc.tile_pool(name="p2tmp" + sfx, bufs=10) as p2tmp, \
         tc.tile_pool(name="p2rb" + sfx, bufs=3) as p2rb, \
         tc.tile_pool(name="wop" + sfx, bufs=1) as wop, \
         tc.tile_pool(name="outp" + sfx, bufs=3) as outp:
        wo_sb = wop.tile([P, HPG, dout], bf, tag="wo")
        nc.sync.dma_start(wo_sb[:], wo[:])

        def emit_po_chain(qt, dsl, dma_eng=None, copy_eng=None):
            po = ps_main.tile([P, SG], f32, tag="ps")
            for h in range(HPG):
                nc.tensor.matmul(
                    po,
                    lhsT=ctx_sb[:, h, qt * P:(qt + 1) * P],
                    rhs=wo_sb[:, h, dsl * SG:(dsl + 1) * SG],
                    start=(h == 0),
                    stop=(h == HPG - 1),
                )
            ob = outp.tile([P, SG], bf, tag="ob")
            if copy_eng is nc.scalar:
                nc.scalar.copy(ob, po)
            else:
                nc.vector.tensor_copy(ob, po)
            (dma_eng or nc.sync).dma_start(
                io["out"][qt * P:(qt + 1) * P, dsl * SG:(dsl + 1) * SG], ob
            )

        # descending g: densest attention first; the previous group's
        # out-proj chains are emitted between each head's scores and PV,
        # exactly where the PE would otherwise stall on the exp latency.
        po_queue = []
        for g in range(nsg - 1, -1, -1):
            qsl = slice(g * SG, (g + 1) * SG)
            jf = 4 * g          # full (below-diagonal) k-tiles
            for h in range(HPG):
                pctx = ps_main.tile([P, SG], f32, tag="ps")

                # stream 1: score MMs + exp. Full tiles in pairs; the 4
                # diagonal tiles packed into two psum tiles with trimmed
                # widths (512,384 | 256,128); tails zeroed so the sum tree
                # is uniform.
                ats = []         # (tile, [(slot, qoff, w), ...])
                for j in range(0, jf, 2):
                    ps2t = ps_sc.tile([P, 2, SG], f32, tag="ps2")
                    for jj in range(2):
                        nc.tensor.matmul(
                            ps2t[:, jj, :],
                            lhsT=kt_sb[:, h, (j + jj) * P:(j + jj + 1) * P],
                            rhs=qt_sb[:, h, qsl],
                            start=True,
                            stop=True,
                        )
                    at2 = p2tmp.tile([P, 2, SG], bf, tag="at")
                    nc.scalar.activation(at2, ps2t, Exp, scale=inv_sqrt_hd)
                    ats.append((at2, [(0, 0, SG), (1, 0, SG)]))
                for pack in ((0, 1), (2, 3)):
                    psd = ps_sc.tile([P, 2, SG], f32, tag="ps2")
                    slots = []
                    for slot, r in enumerate(pack):
                        j = jf + r
                        qoff = r * P
                        w = SG - qoff
                        nc.tensor.matmul(
                            psd[:, slot, 0:w],
                            lhsT=kt_sb[:, h, j * P:(j + 1) * P],
                            rhs=qt_sb[:, h, g * SG + qoff:(g + 1) * SG],
                            start=True,
                            stop=True,
                        )
                        slots.append((slot, qoff, w))
                    dat = p2tmp.tile([P, 2, SG], bf, tag="at")
                    for slot, qoff, w in slots:
                        # write at the global q-offset so all tiles align
                        # column-wise; zero the front for the sum tree
                        nc.scalar.activation(
                            dat[:, slot, qoff:SG], psd[:, slot, 0:w], Exp,
                            scale=inv_sqrt_hd,
                        )
                        if qoff > 0:
                            nc.vector.memset(dat[:, slot, 0:qoff], 0)
                    ats.append((dat, slots))
                # mask the true diagonal 128-block of each trimmed tile
                for dat, slots in ats[-2:]:
                    for slot, qoff, w in slots:
                        nc.vector.tensor_mul(
                            dat[:, slot, qoff:qoff + P],
                            dat[:, slot, qoff:qoff + P], mask_sb
                        )

                # softmax sums (they only need the exps): DVE pair-tree
                # to one tile (diag fronts are zeroed, so full-width adds)
                dsums = []
                for at2, _slots in ats:
                    dsv = p2tmp.tile([P, SG], bf, tag="ds")
                    nc.vector.tensor_add(dsv, at2[:, 0, :], at2[:, 1, :])
                    dsums.append(dsv)
                while len(dsums) > 1:
                    nxt = []
                    for i in range(0, len(dsums) - 1, 2):
                        d2 = p2tmp.tile([P, SG], bf, tag="ds2")
                        nc.vector.tensor_add(d2, dsums[i], dsums[i + 1])
                        nxt.append(d2)
                    if len(dsums) % 2:
                        nxt.append(dsums[-1])
                    dsums = nxt

                # PE filler while ACT drains the exps: previous group's
                # out-proj chains (in-order PE queue, so fillers must
                # come BEFORE the instructions that wait on the exps)
                for _ in range(4):
                    if po_queue:
                        emit_po_chain(*po_queue.pop(0))

                # one ones-matmul -> every partition holds the sum
                psum_l = ps_l.tile([P, SG], f32, tag="l")
                nc.tensor.matmul(
                    psum_l, lhsT=ones_sb, rhs=dsums[0], start=True, stop=True
                )
                rec = p2rb.tile([P, SG], f32, tag="rec")
                nc.vector.reciprocal_approx_fast(rec, psum_l)

                # PV accumulation in j (k-tile) order
                nmm = jf + 4
                mm_i = 0
                for at2, slots in ats:
                    for slot, qoff, w in slots:
                        nc.tensor.matmul(
                            pctx[:, qoff:SG],
                            lhsT=v_sb[:, mm_i, h * P:(h + 1) * P],
                            rhs=at2[:, slot, qoff:SG],
                            start=(mm_i == 0),
                            stop=(mm_i == nmm - 1),
                        )
                        mm_i += 1
                nc.vector.tensor_mul(ctx_sb[:, h, qsl], pctx, rec)

            po_queue = [
                (qt, dsl)
                for qt in range(4 * g, 4 * (g + 1))
                for dsl in range(nos)
            ]
        # final drain: alternate output DMAs across both HWDGE rings and
        # the evacuation copies across DVE/ACT (both idle by now)
        flip = False
        while po_queue:
            emit_po_chain(
                *po_queue.pop(0),
                dma_eng=nc.scalar if flip else nc.sync,
                copy_eng=nc.vector if flip else nc.scalar,
            )
            flip = not flip

    for pool in (ps_l, ps_main, ps_sc, stores, const):
        pool.release()


def build_program(cfg=None):
    import concourse.bacc as bacc
    import concourse.mybir as mybir
    import concourse.tile as tile

    cfg = cfg or {"S": S, "D": D, "DOUT": DOUT}
    bf = mybir.dt.bfloat16
    f32 = mybir.dt.float32
    nc = bacc.Bacc()
    io = {
        "xT": nc.dram_tensor("xT", [cfg["D"], cfg["S"]], bf, kind="ExternalInput"),
        "wq": nc.dram_tensor("wq", [cfg["D"], DG], bf, kind="ExternalInput"),
        "wk": nc.dram_tensor("wk", [cfg["D"], DG], bf, kind="ExternalInput"),
        "wv": nc.dram_tensor("wv", [cfg["D"], DG], bf, kind="ExternalInput"),
        "wo": nc.dram_tensor("wo", [DG, cfg["DOUT"]], bf, kind="ExternalInput"),
        "cosT": nc.dram_tensor("cosT", [P, cfg["S"]], bf, kind="ExternalInput"),
        "sinT": nc.dram_tensor("sinT", [P, cfg["S"]], bf, kind="ExternalInput"),
        "mask": nc.dram_tensor("mask", [P, P], bf, kind="ExternalInput"),
        "ones": nc.dram_tensor("ones", [P, P], bf, kind="ExternalInput"),
        "out": nc.dram_tensor(
            "out", [cfg["S"], cfg["DOUT"]], bf, kind="ExternalOutput"
        ),
    }
    with tile.TileContext(nc) as tc:
        for rep in range(cfg.get("repeat", 1)):
            _emit(tc, io, cfg, sfx=f"_r{rep}")
    nc.finalize()
    return nc


def host_constants(s=S):
    inv = 1.0 / (10000.0 ** (np.arange(0, HEAD, 2, dtype=np.float32) / HEAD))
    pos = np.arange(s, dtype=np.float32)
    ang = pos[:, None] * inv[None, :]
    ang = np.concatenate([ang, ang], axis=-1)          # (s, HEAD)
    cosT = np.cos(ang).T.astype(BF16).copy()           # (HEAD, s)
    sinT = np.sin(ang).T.astype(np.float32)
    sinT[0:64, :] *= -1.0                              # sign-folded rotate-half
    sinT = sinT.astype(BF16).copy()
    kk = np.arange(P)[:, None]
    qq = np.arange(P)[None, :]
    mask = (kk <= qq).astype(BF16)                     # (P, P) tril in [k, q]
    ones = np.ones((P, P), BF16)
    return cosT, sinT, mask, ones


def build_in_maps(x, W_query, W_key, W_value, W_out):
    cosT, sinT, mask, ones = host_constants()
    xTb = [np.ascontiguousarray(np.asarray(x[b]).T).astype(BF16) for b in range(B)]
    in_maps = []
    for core in range(N_CORES):
        b, g = divmod(core, HG)
        gsl = slice(g * DG, (g + 1) * DG)
        in_maps.append({
            "xT": xTb[b],
            "wq": np.asarray(W_query)[:, gsl].astype(BF16).copy(),
            "wk": np.asarray(W_key)[:, gsl].astype(BF16).copy(),
            "wv": np.asarray(W_value)[:, gsl].astype(BF16).copy(),
            "wo": np.asarray(W_out)[gsl, :].astype(BF16).copy(),
            "cosT": cosT, "sinT": sinT, "mask": mask, "ones": ones,
        })
    return in_maps


def gather_out(results, in_dtype=np.float32):
    out = np.zeros((B, S, DOUT), np.float32)
    for core in range(N_CORES):
        out[core // HG] += np.asarray(results[core]["out"], dtype=np.float32)
    return out.astype(in_dtype, copy=False)


def kernel(x, W_query, W_key, W_value, W_out):
    from concourse.bass_utils import run_bass_kernel_spmd

    x = np.asarray(x)
    nc = build_program()
    in_maps = build_in_maps(x, W_query, W_key, W_value, W_out)
    res = run_bass_kernel_spmd(nc, in_maps, core_ids=list(range(N_CORES)))
    return gather_out(res.results, x.dtype)


# revision 40
# speedup vs baseline: 1.1940x; 1.1940x over previous
"""Multi-head causal attention with RoPE on 8 Trainium2 NeuronCores.

Sharding: 2 (batch) x 4 (head-groups of 4 heads). Each core computes
QKV projections, RoPE, flash-style causal attention and its slice of the
output projection for one batch and 4 heads; partial outputs are summed
on the host (row-sharded out_proj => partial-sum reduction).

Device layout choices (everything host-prepped to avoid on-device
transposes, fp32 has no DMA-transpose path):
  - x is passed pre-transposed per batch: xT [D, S] bf16
  - Q^T, K^T computed as [head_dim, S] (lhsT = W tile, rhs = xT)
  - V computed natural [S, head_dim] (lhsT = xT tile, rhs = Wv);
    emitted o-major across 4 parallel si-chains so the PE tracks the
    x DMA stream during startup (no long cold-start stall)
  - scores computed transposed [k, q]; softmax sum over k (partitions)
    via gpsimd partition_all_reduce (lands the sum on every partition,
    so normalization needs no broadcast and no PE ones-matmul)
  - RoPE rotate-half via two SBUF->SBUF partition-swap DMAs plus a
    sign-folded sin constant (no PE permutation matmul)
  - causal diagonal k-tiles use width-trimmed score/PV matmuls
    (only the q >= k columns), one 128x128 tril mask for the true
    diagonal block
  - attention g-outer with per-q-group out-proj interleaved
"""

import math
import sys

import numpy as np

try:
    import concourse.bass as bass  # noqa: F401
except Exception:
    sys.path.insert(0, "/opt/trn_rl_repo")

import ml_dtypes

P = 128
B = 2
S = 2048
D = 2048
H = 16
HEAD = 128
N_CORES = 8
HG = 4            # head groups (tensor-parallel dimension)
HPG = H // HG     # heads per group = 4
DG = HPG * HEAD   # group width = 512
SG = 512          # q-group (free dim) size
DOUT = 2048

BF16 = ml_dtypes.bfloat16


def _emit(tc, io, cfg, sfx=""):
    """Emit the per-core program. io: dict of dram APs. cfg: sizes."""
    import concourse.mybir as mybir
    nc = tc.nc
    bf = mybir.dt.bfloat16
    f32 = mybir.dt.float32
    Exp = mybir.ActivationFunctionType.Exp

    s = cfg["S"]
    d = cfg["D"]
    dout = cfg["DOUT"]
    di_t = d // P          # d_in k-tiles
    st = s // P            # seq 128-tiles
    nsg = s // SG          # seq 512-groups
    nos = dout // SG       # out column slices
    inv_sqrt_hd = 1.0 / math.sqrt(HEAD)

    xT = io["xT"].rearrange("(o p) s -> p o s", p=P)
    wq = io["wq"].rearrange("(o p) n -> p o n", p=P)
    wk = io["wk"].rearrange("(o p) n -> p o n", p=P)
    wv = io["wv"].rearrange("(o p) n -> p o n", p=P)
    wo = io["wo"].rearrange("(o p) n -> p o n", p=P)

    const = tc.alloc_tile_pool(name="const" + sfx, bufs=1)
    stores = tc.alloc_tile_pool(name="stores" + sfx, bufs=1)

    # ---- constants (tiles only; DMAs emitted inside the phase-1 stream) ----
    cos_sb = const.tile([P, s], bf, tag="cos")
    sin_sb = const.tile([P, s], bf, tag="sin")     # sign-folded rope sin
    mask_sb = const.tile([P, P], bf, tag="mask")   # tril (k<=q) diagonal mask
    ones_sb = const.tile([P, P], bf, tag="ones")
    wv_sb = const.tile([P, di_t, DG], bf, tag="wv")

    # persistent activation stores
    qt_sb = stores.tile([P, HPG, s], bf, tag="qt")
    kt_sb = stores.tile([P, HPG, s], bf, tag="kt")
    v_sb = stores.tile([P, st, DG], bf, tag="v")
    ctx_sb = stores.tile([P, HPG, s], bf, tag="ctx")

    # ---- phase 1: projections + RoPE ----
    with tc.tile_pool(name="xt" + sfx, bufs=1) as xtp, \
         tc.tile_pool(name="wqk" + sfx, bufs=1) as wqkp, \
         tc.tile_pool(name="p1tmp" + sfx, bufs=5) as p1tmp, \
         tc.tile_pool(name="p1sw" + sfx, bufs=5) as p1sw, \
         tc.tile_pool(name="ps_v" + sfx, bufs=4, space="PSUM") as ps_v, \
         tc.tile_pool(name="ps_qk" + sfx, bufs=4, space="PSUM") as ps_qk:
        xt_sb = xtp.tile([P, di_t, s], bf, tag="xt")
        wq_sb = wqkp.tile([P, di_t, DG], bf, tag="wq")
        wk_sb = wqkp.tile([P, di_t, DG], bf, tag="wk")
        # x streams as full 524KB o-rows (4KB/partition lines -> near-peak
        # DMA bw); weights as a handful of large DMAs interleaved so the
        # issuing-engine cost (~0.6us per dma_start) stays negligible.
        # first halves of every x row land first: the 8-chain V startup
        # wave only reads s columns 0:1024, so its DMA gate is 4MB not 8MB
        hs = s // 2
        nc.sync.dma_start(wv_sb[:, 0:1, :], wv[:, 0:1, :])
        for o in range(di_t):
            nc.sync.dma_start(xt_sb[:, o, 0:hs], xT[:, o, 0:hs])
            if o == 0:
                nc.sync.dma_start(wv_sb[:, 1:4, :], wv[:, 1:4, :])
            elif o % 4 == 3 and o + 1 < di_t:
                c = o + 1
                nc.sync.dma_start(wv_sb[:, c:c + 4, :], wv[:, c:c + 4, :])
        for o in range(di_t):
            nc.sync.dma_start(xt_sb[:, o, hs:s], xT[:, o, hs:s])
        nc.sync.dma_start(cos_sb[:], io["cosT"][:])
        nc.sync.dma_start(sin_sb[:], io["sinT"][:])
        nc.sync.dma_start(mask_sb[:], io["mask"][:])
        nc.sync.dma_start(ones_sb[:], io["ones"][:])
        nc.sync.dma_start(wq_sb[:], wq[:])
        nc.sync.dma_start(wk_sb[:], wk[:])

        # V natural layout [s_tile, DG]. First 8 si-chains run o-major in
        # parallel (all 8 PSUM banks) so the PE tracks the x o-row DMA
        # stream during startup; the rest run chain-at-a-time so
        # evacuations overlap.
        pvs = [
            (ps_v if i < 4 else ps_qk).tile(
                [P, SG], f32, tag="psv" if i < 4 else "ps", name=f"pv0_{i}"
            )
            for i in range(8)
        ]
        for o in range(di_t):
            for q8 in range(8):
                nc.tensor.matmul(
                    pvs[q8][:, :DG],
                    lhsT=xt_sb[:, o, q8 * P:(q8 + 1) * P],
                    rhs=wv_sb[:, o, :],
                    start=(o == 0),
                    stop=(o == di_t - 1),
                )
        for q8 in range(8):
            nc.vector.tensor_copy(v_sb[:, q8, :], pvs[q8][:, :DG])
        for si in range(8, st):
            pv = ps_v.tile([P, SG], f32, tag="psv")
            for o in range(di_t):
                nc.tensor.matmul(
                    pv[:, :DG],
                    lhsT=xt_sb[:, o, si * P:(si + 1) * P],
                    rhs=wv_sb[:, o, :],
                    start=(o == 0),
                    stop=(o == di_t - 1),
                )
            nc.vector.tensor_copy(v_sb[:, si, :], pv[:, :DG])

        # Q^T, K^T with RoPE, per head. Rotate-half = partition swap via
        # two SBUF->SBUF DMAs + sign-folded sin; pipelined 2 deep so the
        # PE never waits on the ACT psum->sbuf copy or the swap DMA.
        def emit_rope(qa, qsw, dst, hh, sl):
            t1 = p1tmp.tile([P, SG], bf, tag="t1")
            nc.vector.tensor_mul(t1, qa, cos_sb[:, sl])
            t2 = p1tmp.tile([P, SG], bf, tag="t2")
            nc.vector.tensor_mul(t2, qsw, sin_sb[:, sl])
            nc.vector.tensor_add(dst[:, hh, sl], t1, t2)

        pending = []
        for h in range(HPG):
            hsl = slice(h * P, (h + 1) * P)
            for g in range(nsg):
                sl = slice(g * SG, (g + 1) * SG)
                for w_t, dst in ((wq_sb, qt_sb), (wk_sb, kt_sb)):
                    pq = ps_qk.tile([P, SG], f32, tag="ps")
                    for o in range(di_t):
                        nc.tensor.matmul(
                            pq,
                            lhsT=w_t[:, o, hsl],
                            rhs=xt_sb[:, o, sl],
                            start=(o == 0),
                            stop=(o == di_t - 1),
                        )
                    qa = p1tmp.tile([P, SG], bf, tag="qa")
                    nc.scalar.copy(qa, pq)
                    qsw = p1sw.tile([P, SG], bf, tag="qsw")
                    nc.sync.dma_start(qsw[0:64, :], qa[64:128, :])
                    nc.sync.dma_start(qsw[64:128, :], qa[0:64, :])
                    pending.append((qa, qsw, dst, h, sl))
                    while len(pending) > 2:
                        emit_rope(*pending.pop(0))
        while pending:
            emit_rope(*pending.pop(0))

    # ---- phase 2+3: attention interleaved with output projection ----
    # g outer so each q-group's out-proj tiles become ready early and fill
    # the PE while later q-groups' softmax runs. Diagonal k-tiles use
    # width-trimmed matmuls; softmax sums pair-tree on DVE into one tile,
    # then a single full-width ones-matmul per (h, g) lands the sum on
    # every PSUM partition so normalization needs no broadcast.
    ps_sc = tc.alloc_tile_pool(name="ps_sc" + sfx, bufs=2, space="PSUM")
    ps_main = tc.alloc_tile_pool(name="ps_main" + sfx, bufs=3, space="PSUM")
    ps_l = tc.alloc_tile_pool(name="ps_l" + sfx, bufs=1, space="PSUM")
    with tc.tile_pool(name="p2tmp" + sfx, bufs=10) as p2tmp, \
         tc.tile_pool(name="p2rb" + sfx, bufs=3) as p2rb, \
         tc.tile_pool(name="wop" + sfx, bufs=1) as wop, \
         tc.tile_pool(name="outp" + sfx, bufs=3) as outp:
        wo_sb = wop.tile([P, HPG, dout], bf, tag="wo")
        nc.sync.dma_start(wo_sb[:], wo[:])

        def emit_po_chain(qt, dsl, dma_eng=None, copy_eng=None):
            po = ps_main.tile([P, SG], f32, tag="ps")
            for h in range(HPG):
                nc.tensor.matmul(
                    po,
                    lhsT=ctx_sb[:, h, qt * P:(qt + 1) * P],
                    rhs=wo_sb[:, h, dsl * SG:(dsl + 1) * SG],
                    start=(h == 0),
                    stop=(h == HPG - 1),
                )
            ob = outp.tile([P, SG], bf, tag="ob")
            if copy_eng is nc.scalar:
                nc.scalar.copy(ob, po)
            else:
                nc.vector.tensor_copy(ob, po)
            (dma_eng or nc.sync).dma_start(
                io["out"][qt * P:(qt + 1) * P, dsl * SG:(dsl + 1) * SG], ob
            )

        # descending g: densest attention first; the previous group's
        # out-proj chains are emitted between each head's scores and PV,
        # exactly where the PE would otherwise stall on the exp latency.
        po_queue = []
        for g in range(nsg - 1, -1, -1):
            qsl = slice(g * SG, (g + 1) * SG)
            jf = 4 * g          # full (below-diagonal) k-tiles
            for h in range(HPG):
                pctx = ps_main.tile([P, SG], f32, tag="ps")

                # stream 1: score MMs + exp. Full tiles in pairs; the 4
                # diagonal tiles packed into two psum tiles with trimmed
                # widths (512,384 | 256,128); tails zeroed so the sum tree
                # is uniform.
                ats = []         # (tile, [(slot, qoff, w), ...])
                for j in range(0, jf, 2):
                    ps2t = ps_sc.tile([P, 2, SG], f32, tag="ps2")
                    for jj in range(2):
                        nc.tensor.matmul(
                            ps2t[:, jj, :],
                            lhsT=kt_sb[:, h, (j + jj) * P:(j + jj + 1) * P],
                            rhs=qt_sb[:, h, qsl],
                            start=True,
                            stop=True,
                        )
                    at2 = p2tmp.tile([P, 2, SG], bf, tag="at")
                    nc.scalar.activation(at2, ps2t, Exp, scale=inv_sqrt_hd)
                    ats.append((at2, [(0, 0, SG), (1, 0, SG)]))
                for pack in ((0, 1), (2, 3)):
                    psd = ps_sc.tile([P, 2, SG], f32, tag="ps2")
                    slots = []
                    for slot, r in enumerate(pack):
                        j = jf + r
                        qoff = r * P
                        w = SG - qoff
                        nc.tensor.matmul(
                            psd[:, slot, 0:w],
                            lhsT=kt_sb[:, h, j * P:(j + 1) * P],
                            rhs=qt_sb[:, h, g * SG + qoff:(g + 1) * SG],
                            start=True,
                            stop=True,
                        )
                        slots.append((slot, qoff, w))
                    dat = p2tmp.tile([P, 2, SG], bf, tag="at")
                    for slot, qoff, w in slots:
                        # write at the global q-offset so all tiles align
                        # column-wise; zero the front for the sum tree
                        nc.scalar.activation(
                            dat[:, slot, qoff:SG], psd[:, slot, 0:w], Exp,
                            scale=inv_sqrt_hd,
                        )
                        if qoff > 0:
                            nc.vector.memset(dat[:, slot, 0:qoff], 0)
                    ats.append((dat, slots))
                # mask the true diagonal 128-block of each trimmed tile
                for dat, slots in ats[-2:]:
                    for slot, qoff, w in slots:
                        nc.vector.tensor_mul(
                            dat[:, slot, qoff:qoff + P],
                            dat[:, slot, qoff:qoff + P], mask_sb
                        )

                # softmax sums (they only need the exps): DVE pair-tree
                # to one tile (diag fronts are zeroed, so full-width adds)
                dsums = []
                for at2, _slots in ats:
                    dsv = p2tmp.tile([P, SG], bf, tag="ds")
                    nc.vector.tensor_add(dsv, at2[:, 0, :], at2[:, 1, :])
                    dsums.append(dsv)
                while len(dsums) > 1:
                    nxt = []
                    for i in range(0, len(dsums) - 1, 2):
                        d2 = p2tmp.tile([P, SG], bf, tag="ds2")
                        nc.vector.tensor_add(d2, dsums[i], dsums[i + 1])
                        nxt.append(d2)
                    if len(dsums) % 2:
                        nxt.append(dsums[-1])
                    dsums = nxt

                # PE filler while ACT drains the exps: previous group's
                # out-proj chains (in-order PE queue, so fillers must
                # come BEFORE the instructions that wait on the exps)
                for _ in range(4):
                    if po_queue:
                        emit_po_chain(*po_queue.pop(0))

                # one ones-matmul -> every partition holds the sum
                psum_l = ps_l.tile([P, SG], f32, tag="l")
                nc.tensor.matmul(
                    psum_l, lhsT=ones_sb, rhs=dsums[0], start=True, stop=True
                )
                rec = p2rb.tile([P, SG], f32, tag="rec")
                nc.vector.reciprocal_approx_fast(rec, psum_l)

                # PV accumulation in j (k-tile) order
                nmm = jf + 4
                mm_i = 0
                for at2, slots in ats:
                    for slot, qoff, w in slots:
                        nc.tensor.matmul(
                            pctx[:, qoff:SG],
                            lhsT=v_sb[:, mm_i, h * P:(h + 1) * P],
                            rhs=at2[:, slot, qoff:SG],
                            start=(mm_i == 0),
                            stop=(mm_i == nmm - 1),
                        )
                        mm_i += 1
                nc.vector.tensor_mul(ctx_sb[:, h, qsl], pctx, rec)

            po_queue = [
                (qt, dsl)
                for qt in range(4 * g, 4 * (g + 1))
                for dsl in range(nos)
            ]
        # final drain: alternate output DMAs across both HWDGE rings and
        # the evacuation copies across DVE/ACT (both idle by now)
        flip = False
        while po_queue:
            emit_po_chain(
                *po_queue.pop(0),
                dma_eng=nc.scalar if flip else nc.sync,
                copy_eng=nc.vector if flip else nc.scalar,
            )
            flip = not flip

    for pool in (ps_l, ps_main, ps_sc, stores, const):
        pool.release()


def build_program(cfg=None):
    import concourse.bacc as bacc
    import concourse.mybir as mybir
    import concourse.tile as tile

    cfg = cfg or {"S": S, "D": D, "DOUT": DOUT}
    bf = mybir.dt.bfloat16
    f32 = mybir.dt.float32
    nc = bacc.Bacc()
    io = {
        "xT": nc.dram_tensor("xT", [cfg["D"], cfg["S"]], bf, kind="ExternalInput"),
        "wq": nc.dram_tensor("wq", [cfg["D"], DG], bf, kind="ExternalInput"),
        "wk": nc.dram_tensor("wk", [cfg["D"], DG], bf, kind="ExternalInput"),
        "wv": nc.dram_tensor("wv", [cfg["D"], DG], bf, kind="ExternalInput"),
        "wo": nc.dram_tensor("wo", [DG, cfg["DOUT"]], bf, kind="ExternalInput"),
        "cosT": nc.dram_tensor("cosT", [P, cfg["S"]], bf, kind="ExternalInput"),
        "sinT": nc.dram_tensor("sinT", [P, cfg["S"]], bf, kind="ExternalInput"),
        "mask": nc.dram_tensor("mask", [P, P], bf, kind="ExternalInput"),
        "ones": nc.dram_tensor("ones", [P, P], bf, kind="ExternalInput"),
        "out": nc.dram_tensor(
            "out", [cfg["S"], cfg["DOUT"]], bf, kind="ExternalOutput"
        ),
    }
    with tile.TileContext(nc) as tc:
        for rep in range(cfg.get("repeat", 1)):
            _emit(tc, io, cfg, sfx=f"_r{rep}")
    nc.finalize()
    return nc


def host_constants(s=S):
    inv = 1.0 / (10000.0 ** (np.arange(0, HEAD, 2, dtype=np.float32) / HEAD))
    pos = np.arange(s, dtype=np.float32)
    ang = pos[:, None] * inv[None, :]
    ang = np.concatenate([ang, ang], axis=-1)          # (s, HEAD)
    cosT = np.cos(ang).T.astype(BF16).copy()           # (HEAD, s)
    sinT = np.sin(ang).T.astype(np.float32)
    sinT[0:64, :] *= -1.0                              # sign-folded rotate-half
    sinT = sinT.astype(BF16).copy()
    kk = np.arange(P)[:, None]
    qq = np.arange(P)[None, :]
    mask = (kk <= qq).astype(BF16)                     # (P, P) tril in [k, q]
    ones = np.ones((P, P), BF16)
    return cosT, sinT, mask, ones


def build_in_maps(x, W_query, W_key, W_value, W_out):
    cosT, sinT, mask, ones = host_constants()
    xTb = [np.ascontiguousarray(np.asarray(x[b]).T).astype(BF16) for b in range(B)]
    in_maps = []
    for core in range(N_CORES):
        b, g = divmod(core, HG)
        gsl = slice(g * DG, (g + 1) * DG)
        in_maps.append({
            "xT": xTb[b],
            "wq": np.asarray(W_query)[:, gsl].astype(BF16).copy(),
            "wk": np.asarray(W_key)[:, gsl].astype(BF16).copy(),
            "wv": np.asarray(W_value)[:, gsl].astype(BF16).copy(),
            "wo": np.asarray(W_out)[gsl, :].astype(BF16).copy(),
            "cosT": cosT, "sinT": sinT, "mask": mask, "ones": ones,
        })
    return in_maps


def gather_out(results, in_dtype=np.float32):
    out = np.zeros((B, S, DOUT), np.float32)
    for core in range(N_CORES):
        out[core // HG] += np.asarray(results[core]["out"], dtype=np.float32)
    return out.astype(in_dtype, copy=False)


def kernel(x, W_query, W_key, W_value, W_out):
    from concourse.bass_utils import run_bass_kernel_spmd

    x = np.asarray(x)
    nc = build_program()
    in_maps = build_in_maps(x, W_query, W_key, W_value, W_out)
    res = run_bass_kernel_spmd(nc, in_maps, core_ids=list(range(N_CORES)))
    return gather_out(res.results, x.dtype)
